# revision 25
# baseline (speedup 1.0000x reference)
"""Spectral pooling (FFT2 -> crop low freqs -> IFFT2) as dense DFT matmuls
on TRN2, batch-sharded across 8 NeuronCores (4 batches/core, no comms).

Input  x: (32, 256, 64, 64) fp32 -- channels 0:128 real part, 128:256 imag.
Output y: (32, 256, 32, 32) fp32.  Math: per complex image, Y = A @ X @ A.T
with A = orthonormal IDFT32 @ Crop @ DFT64 (32x64 complex).

Design (each piece HW-A/B-validated):
  - Host pre-packs x to bf16 with partitions=(ri, h): real AND imag parts of
    each complex channel stacked on the partition dim, so stage-1 does the
    complex H-transform in ONE dense matmul (K=128=(ri,h), N=64=(riout,fh)).
    Halves HBM load traffic vs fp32 and makes loads fully contiguous.
  - Stage-2 contracts (c2, w) with block-diagonal rhs; the complex combine
    is a 2-matmul PSUM accumulation. 96 matmuls / 8192 PE cycles per batch.
  - PSUM -> SBUF copies batched to [128, 512]/[128, 256] (per-instruction
    init dominates copies; the partition dim is free), split DVE/ACT.
  - mega I/O: ONE 8 MiB load + ONE 2 MiB store per 4-batch pass -- real HBM
    DMA throughput rises steeply with transfer size (~341 GB/s at 1 MiB vs
    ~425 at 8 MiB; the biggest single win, ~33 -> ~25 us).
  - Output stored bf16 in device layout; host restores fp32 + reference
    layout (error budget: ~5.5e-3 total vs the 2e-2 gate).
"""

import math

import numpy as np

from concourse import bass, mybir
from concourse.bass_utils import run_bass_kernel_spmd
from concourse.tile import TileContext

N_CORES = 8
B_FULL, C2, H, W = 32, 256, 64, 64
HP, WP = 32, 32
BPC = B_FULL // N_CORES

F32 = mybir.dt.float32
BF16 = mybir.dt.bfloat16


def _split_multi_waits(nc):
    """Hoist extra semaphore waits onto same-engine NOPs (this walrus build
    rejects instructions carrying more than one wait)."""
    n_split = 0
    for f in nc.m.functions:
        for bb in f.blocks:
            insts = bb.instructions
            out = []
            for inst in insts:
                si = inst.sync_info
                waits = list(si.on_wait) if si and si.on_wait else []
                if len(waits) > 1:
                    si.on_wait = waits[-1:]
                    for w in waits[:-1]:
                        nop = mybir.InstNoOp(
                            name=nc.get_next_instruction_name(),
                            ins=[],
                            outs=[],
                            engine=inst.engine,
                            sync_info=mybir.SyncInfo(on_wait=[w], on_update=[]),
                        )
                        out.append(nop)
                        n_split += 1
                out.append(inst)
            if len(out) != len(insts):
                insts[:] = out
    return n_split


def _a_matrix():
    topf = int(math.ceil(H * 0.5 / 2))  # 16
    midf = H // 2 + topf  # 48
    F = np.exp(-2j * np.pi * np.outer(np.arange(H), np.arange(H)) / H)
    G = np.exp(2j * np.pi * np.outer(np.arange(HP), np.arange(HP)) / HP)
    keep = list(range(topf)) + list(range(midf, H))
    S = np.zeros((HP, H))
    S[np.arange(HP), keep] = 1
    return (G @ S @ F) / np.sqrt(H * W * HP * WP) ** 0.5  # [32, 64] complex


def _dft_constants():
    """[3, 128, 128] fp32: E1 (cols 0:64 used), F2r, F2i."""
    A = _a_matrix()
    Ar = A.real.astype(np.float32)  # [32, 64]
    Ai = A.imag.astype(np.float32)

    E1 = np.zeros((128, 128), np.float32)
    # rows (ri, h); cols (riout, fh)
    E1[0:64, 0:32] = Ar.T
    E1[64:128, 0:32] = -Ai.T
    E1[0:64, 32:64] = Ai.T
    E1[64:128, 32:64] = Ar.T

    F2r = np.zeros((128, 128), np.float32)
    F2i = np.zeros((128, 128), np.float32)
    # rows (c2, w); cols (c2, riY, fw); block-diagonal in c2
    for c2 in range(2):
        r0, c0 = 64 * c2, 64 * c2
        F2r[r0 : r0 + 64, c0 : c0 + 32] = Ar.T
        F2r[r0 : r0 + 64, c0 + 32 : c0 + 64] = Ai.T
        F2i[r0 : r0 + 64, c0 : c0 + 32] = -Ai.T
        F2i[r0 : r0 + 64, c0 + 32 : c0 + 64] = Ar.T
    return np.stack([E1, F2r, F2i])


def _pack_x(x):
    """(B, 256, 64, 64) fp32 -> (B, 128, 8192) bf16.

    partition p = 64*ri + h ; free f = 128*cp + 64*c2 + w
    complex channel cc = 2*cp + c2; x channel = 128*ri + cc.
    """
    b = x.shape[0]
    xb = x.reshape(b, 2, 64, 2, 64, 64)  # b ri cp c2 h w
    xb = xb.transpose(0, 1, 4, 2, 3, 5)  # b ri h cp c2 w
    return np.ascontiguousarray(xb).reshape(b, 128, 8192).astype(
        mybir.dt.np(BF16))


def _unpack_y(y_dev):
    """(B, 128, 2048) bf16 -> (B, 256, 32, 32) fp32.

    y[b, 128*riY + cc, fh, fw] = y_dev[b, 32*cp4 + fh,
        256*G8 + 128*cphalf + 64*c2 + 32*riY + fw]
    with cc = 16*G8 + 8*cphalf + 2*cp4 + c2.
    """
    b = y_dev.shape[0]
    yv = y_dev.astype(np.float32).reshape(b, 4, 32, 8, 2, 2, 2, 32)
    # b cp4 fh G8 cphalf c2 riY fw
    yv = yv.transpose(0, 6, 3, 4, 1, 5, 2, 7)  # b riY G8 cphalf cp4 c2 fh fw
    return np.ascontiguousarray(yv).reshape(b, 256, 32, 32)


def build_program(reps: int = 1, split_waits: bool = True, skew: int = 2,
                  mega: bool = True, mega_splits: int = 2):
    nc = bass.Bass("TRN2", target_bir_lowering=False, debug=False)
    x = nc.dram_tensor("x", [BPC, 128, 8192], BF16, kind="ExternalInput").ap()
    dm = nc.dram_tensor("dmats", [3, 128, 128], F32, kind="ExternalInput").ap()
    y = nc.dram_tensor("y", [BPC, 128, 2048], BF16, kind="ExternalOutput").ap()

    NG = 8  # stage groups per batch; group = 8 consecutive cp (16 channels)

    with TileContext(nc) as tc:
        with (
            tc.tile_pool(name="consts", bufs=1) as cpool,
            tc.tile_pool(name="inp", bufs=2 if mega else 3) as ipool,
            tc.tile_pool(name="sb1", bufs=2 * (skew + 1)) as s1pool,
            tc.tile_pool(name="sbout", bufs=2 if mega else 3) as opool,
            tc.tile_pool(name="ps1", bufs=4, space="PSUM") as p1pool,
            tc.tile_pool(name="ps2", bufs=3, space="PSUM") as p2pool,
        ):
            dmf = cpool.tile([128, 384], F32, tag="dmf")
            dmb = cpool.tile([128, 384], BF16, tag="dmb")
            for k in range(3):
                nc.sync.dma_start(out=dmf[:, 128 * k : 128 * (k + 1)], in_=dm[k])
            nc.vector.tensor_copy(out=dmb, in_=dmf)
            e1b = dmb[:, 0:64]
            f2rb = dmb[:, 128:256]
            f2ib = dmb[:, 256:384]

            in_tiles = {}
            out_tiles = {}
            cp_state = [0]
            mega_state = {}

            def copy_eng():
                cp_state[0] ^= 1
                return nc.vector if cp_state[0] else nc.scalar

            def copy(eng, out, in_):
                if eng is nc.vector:
                    nc.vector.tensor_copy(out=out, in_=in_)
                else:
                    nc.scalar.copy(out=out, in_=in_)

            def stage1(b, g):
                if mega and b not in in_tiles:
                    if b == 0:
                        # 8 MiB of loads per rep in mega_splits DMAs: HBM
                        # DMA throughput rises steeply with transfer size,
                        # but splitting lets compute start earlier
                        in_all = ipool.tile([128, 32768], BF16, tag="in_all")
                        iav = in_all.rearrange(
                            "p (bb f) -> p bb f", bb=BPC, f=8192)
                        xv = x.transpose([1, 0, 2])
                        bstep = BPC // mega_splits
                        for ms in range(mega_splits):
                            bs = slice(bstep * ms, bstep * (ms + 1))
                            nc.sync.dma_start(out=iav[:, bs], in_=xv[:, bs])
                        sb_all = opool.tile([128, 8192], BF16, tag="sb_all")
                        mega_state["in"] = in_all
                        mega_state["out"] = sb_all
                    in_tiles[b] = mega_state["in"][
                        :, 8192 * b : 8192 * (b + 1)]
                    out_tiles[b] = mega_state["out"][
                        :, 2048 * b : 2048 * (b + 1)]
                if b not in in_tiles:
                    in_t = ipool.tile([128, 8192], BF16, tag="in_t")
                    # quarter-split loads matching group consumption order
                    # (group g reads cols 1024*g : 1024*(g+1))
                    for qt in range(4):
                        cs = slice(2048 * qt, 2048 * (qt + 1))
                        nc.sync.dma_start(out=in_t[:, cs], in_=x[b, :, cs])
                    in_tiles[b] = in_t
                    out_tiles[b] = opool.tile(
                        [128, 2048], BF16, tag="sb_out", name=f"sb_out_{b}")
                in_t = in_tiles[b]
                # psum1 cols laid out (riout, cphalf, cp4, fh): each matmul
                # scatters its (riout, fh) output into two 32-col runs so the
                # PSUM -> SBUF copy is a plain contiguous [128, 512] op and
                # stage-2 lhsT slices are contiguous.
                psum1 = p1pool.tile([128, 512], F32, tag="ps1")
                pview = psum1.rearrange(
                    "p (r j f) -> p j r f", r=2, j=8, f=32)
                for j in range(8):
                    cp = 8 * g + j
                    nc.tensor.matmul(
                        out=pview[:, j],
                        lhsT=in_t[:, 128 * cp : 128 * (cp + 1)],
                        rhs=e1b, start=True, stop=True,
                        tile_position=(0, 0),
                    )
                sb1 = s1pool.tile([128, 512], BF16, tag="sb1")
                copy(copy_eng(), sb1, psum1)
                return sb1

            def stage2(b, g, sb1):
                sb_out = out_tiles[b]
                psum2 = p2pool.tile([128, 256], F32, tag="ps2")
                for ch in range(2):
                    co = slice(128 * ch, 128 * (ch + 1))
                    nc.tensor.matmul(
                        out=psum2[:, co],
                        lhsT=sb1[:, 128 * ch : 128 * (ch + 1)],
                        rhs=f2rb, start=True, stop=False,
                        tile_position=(0, 0),
                    )
                    nc.tensor.matmul(
                        out=psum2[:, co],
                        lhsT=sb1[:, 256 + 128 * ch : 256 + 128 * (ch + 1)],
                        rhs=f2ib, start=False, stop=True,
                        tile_position=(0, 0),
                    )
                copy(copy_eng(), sb_out[:, 256 * g : 256 * (g + 1)], psum2)
                if g == NG - 1:
                    del in_tiles[b]
                    del out_tiles[b]
                    if mega:
                        bstep = BPC // mega_splits
                        if (b + 1) % bstep == 0:
                            bs = slice(b + 1 - bstep, b + 1)
                            nc.scalar.dma_start(
                                out=y.transpose([1, 0, 2])[:, bs],
                                in_=mega_state["out"].rearrange(
                                    "p (bb c) -> p bb c", bb=BPC, c=2048,
                                )[:, bs],
                            )
                    else:
                        nc.scalar.dma_start(out=y[b], in_=sb_out)

            work = [(b, g)
                    for _ in range(reps)
                    for b in range(BPC)
                    for g in range(NG)]
            pend = []
            for b, g in work:
                pend.append((b, g, stage1(b, g)))
                if len(pend) > skew:
                    stage2(*pend.pop(0))
            while pend:
                stage2(*pend.pop(0))
    if split_waits:
        _split_multi_waits(nc)
    return nc


_CACHED = {}


def _get_program():
    if "nc" not in _CACHED:
        _CACHED["nc"] = build_program()
        _CACHED["consts"] = _dft_constants()
    return _CACHED["nc"], _CACHED["consts"]


def kernel(x: np.ndarray) -> np.ndarray:
    assert x.shape == (B_FULL, C2, H, W) and x.dtype == np.float32
    nc, dmats = _get_program()
    xp = _pack_x(np.ascontiguousarray(x))
    in_maps = [
        {"x": xp[BPC * k : BPC * (k + 1)], "dmats": dmats}
        for k in range(N_CORES)
    ]
    res = run_bass_kernel_spmd(nc, in_maps, list(range(N_CORES)))
    y_dev = np.concatenate(
        [res.results[k]["y"] for k in range(N_CORES)], axis=0
    )
    return _unpack_y(y_dev).astype(np.float32, copy=False)


if __name__ == "__main__":
    rng = np.random.default_rng(0)
    x = rng.standard_normal((B_FULL, C2, H, W)).astype(np.float32)
    y = kernel(x)
    print("kernel output", y.shape, y.dtype)
